# revision 5
# baseline (speedup 1.0000x reference)
"""ExpertGroupRouter MoE routing kernel for 8 TRN2 NeuronCores (Bass/Tile).

Strategy (data-parallel over tokens):
  - Flatten x to (16384, 2048) tokens; shard 2048 contiguous tokens per core.
  - Per core: stream x in 16 tiles of (128 tok, 2048 d). PE-transpose each
    128x128 block to build xT (d-major) chunks, copy PSUM->SBUF (DVE/ACT
    alternating), then PE matmul with the replicated 18-col weight matrix
    W^T (W_expert ++ W_group) to get scoresT (18, tok) with f32 PSUM
    accumulation over the 16 d-chunks.  float32r (11-bit-mantissa fp32) is
    used on the PE for 4x matmul and ~2.5x transpose throughput; the
    routing decisions stay f32.
  - scoresT is PE-transposed back to token-major (128, 18) tiles and the
    full routing logic (grouped softmax top-1/top-1/top-2, sigmoid gates,
    weight normalization, expert bincount) runs batched on DVE/ACT.
  - Host gathers the 8 shards, sums the per-core/per-partition expert
    counts and computes the scalar KL aux loss from the 16 totals.
"""
import sys

if "/opt/trn_rl_repo" not in sys.path:
    sys.path.insert(0, "/opt/trn_rl_repo")

import numpy as np

B, T, D, E = 4, 4096, 2048, 16
N_CORES = 8
TOK = B * T                  # 16384 tokens
TPC = TOK // N_CORES         # 2048 tokens per core
N_TILES = TPC // 128         # 16 token tiles per core
N_CHUNKS = D // 128          # 16 d-chunks
GROUPS = N_TILES // 4        # 4 groups of 512 tokens
BIG = 65536.0
THR = 0.15

_cache = {}


def _round_f32r(a: np.ndarray) -> np.ndarray:
    """Round f32 bits to the PE's fp32r format (RNE to 11 mantissa bits)."""
    u = a.astype(np.float32).view(np.uint32).astype(np.uint64)
    u = (u + 0x7FF + ((u >> 12) & 1)) & 0xFFFFF000
    return (u & 0xFFFFFFFF).astype(np.uint32).view(np.float32)


def _build():
    if "nc" in _cache:
        return _cache["nc"]
    import concourse.bass as bass
    import concourse.tile as tile
    from concourse import mybir, bacc

    F32, F32R, I32 = mybir.dt.float32, mybir.dt.float32r, mybir.dt.int32
    AX = mybir.AxisListType
    OP = mybir.AluOpType
    ACTF = mybir.ActivationFunctionType

    nc = bacc.Bacc("TRN2", target_bir_lowering=False, debug=False)
    x_d = nc.dram_tensor("x", [TPC, D], F32R, kind="ExternalInput")
    wT_d = nc.dram_tensor("wT", [D, 18], F32R, kind="ExternalInput")
    id_d = nc.dram_tensor("ident", [128, 128], F32R, kind="ExternalInput")
    iota_d = nc.dram_tensor("iota", [128, 16 * 16], F32, kind="ExternalInput")
    ow_d = nc.dram_tensor("out_w", [128, N_TILES * 6], F32, kind="ExternalOutput")
    oi_d = nc.dram_tensor("out_i", [128, N_TILES * 6], I32, kind="ExternalOutput")
    oc_d = nc.dram_tensor("out_c", [128, E], F32, kind="ExternalOutput")

    import contextlib

    with tile.TileContext(nc) as tc, contextlib.ExitStack() as ctx:
        const = ctx.enter_context(tc.tile_pool(name="const", bufs=1))
        xpool = ctx.enter_context(tc.tile_pool(name="xpool", bufs=8))
        xtp = ctx.enter_context(tc.tile_pool(name="xtp", bufs=3))
        pxT = ctx.enter_context(tc.tile_pool(name="pxT", bufs=2, space="PSUM"))
        psc = ctx.enter_context(tc.tile_pool(name="psc", bufs=2, space="PSUM"))
        pbt = ctx.enter_context(tc.tile_pool(name="pbt", bufs=2, space="PSUM"))
        sc_sb = ctx.enter_context(tc.tile_pool(name="sc_sb", bufs=2))
        rt = ctx.enter_context(tc.tile_pool(name="rt", bufs=1))

        # ---- constants ----
        wTt = const.tile([128, N_CHUNKS * 18], F32R)
        nc.sync.dma_start(
            wTt[:].rearrange("p (k e) -> p k e", k=N_CHUNKS),
            wT_d.ap().rearrange("(k p) e -> p k e", p=128),
        )
        idt = const.tile([128, 128], F32R)
        nc.sync.dma_start(idt[:], id_d.ap()[:])
        iot = const.tile([128, 16 * 16], F32)
        nc.sync.dma_start(iot[:], iota_d.ap()[:])

        # persistent routing tiles
        S_all = rt.tile([128, N_TILES * 18], F32)   # token-major scores

        # ---- main streaming loop: scores ----
        for g in range(GROUPS):
            xt = []
            for tt in range(4):
                t = xpool.tile([128, D], F32R, tag="xt")
                r0 = (g * 4 + tt) * 128
                nc.sync.dma_start(t[:], x_d.ap()[r0:r0 + 128, :])
                xt.append(t)

            po = psc.tile([18, 512], F32, tag="po")
            for k in range(N_CHUNKS):
                pt = pxT.tile([128, 512], F32R, tag="pt")
                for tt in range(4):
                    nc.tensor.transpose(
                        pt[:, tt * 128:(tt + 1) * 128],
                        xt[tt][:, k * 128:(k + 1) * 128],
                        idt[:],
                    )
                st = xtp.tile([128, 512], F32R, tag="st")
                if k % 2 == 0:
                    nc.vector.tensor_copy(st[:], pt[:])
                else:
                    nc.scalar.copy(st[:], pt[:])
                nc.tensor.matmul(
                    po[:], wTt[:, k * 18:(k + 1) * 18], st[:],
                    start=(k == 0), stop=(k == N_CHUNKS - 1),
                )

            # scoresT (18, 512) -> token-major (128, 18) tiles
            sct = sc_sb.tile([18, 512], F32, tag="sct")
            nc.scalar.copy(sct[:], po[:])
            for tt in range(4):
                pb = pbt.tile([128, 18], F32, tag="pb")
                nc.tensor.transpose(
                    pb[:], sct[:, tt * 128:(tt + 1) * 128],
                    idt[0:18, 0:18].bitcast(F32),
                )
                dst = S_all[:, (g * 4 + tt) * 18:(g * 4 + tt + 1) * 18]
                nc.vector.tensor_copy(dst, pb[:])

        # ---- batched routing on (128, 16 tiles, 18) ----
        S = S_all[:].rearrange("p (t e) -> p t e", t=N_TILES)
        NT = N_TILES

        def bcast(t2d, w):
            # (128, NT) -> (128, NT, w) stride-0 broadcast
            return t2d[:].to_broadcast([128, NT, w])

        sa, sb4, sc4 = S[:, :, 0:8], S[:, :, 8:12], S[:, :, 12:16]
        gsc = S[:, :, 16:18]
        io3 = iot[:].rearrange("p (t e) -> p t e", t=16)
        ioA, ioB, ioC = io3[:, :, 0:8], io3[:, :, 8:12], io3[:, :, 12:16]

        m_a = rt.tile([128, NT], F32)
        m_b = rt.tile([128, NT], F32)
        m_c = rt.tile([128, NT], F32)
        nc.vector.reduce_max(m_a[:], sa, axis=AX.X)
        nc.vector.reduce_max(m_b[:], sb4, axis=AX.X)
        nc.vector.reduce_max(m_c[:], sc4, axis=AX.X)

        E_all = rt.tile([128, NT * 16], F32)
        nc.scalar.activation(
            E_all[:].rearrange("p (t e) -> p t e", t=NT), S[:, :, 0:16], ACTF.Exp
        )
        E3 = E_all[:].rearrange("p (t e) -> p t e", t=NT)
        sum_a = rt.tile([128, NT], F32)
        sum_b = rt.tile([128, NT], F32)
        sum_c = rt.tile([128, NT], F32)
        nc.vector.reduce_sum(sum_a[:], E3[:, :, 0:8], axis=AX.X)
        nc.vector.reduce_sum(sum_b[:], E3[:, :, 8:12], axis=AX.X)
        nc.vector.reduce_sum(sum_c[:], E3[:, :, 12:16], axis=AX.X)

        em_a = rt.tile([128, NT], F32)
        em_b = rt.tile([128, NT], F32)
        em_c = rt.tile([128, NT], F32)
        nc.scalar.activation(em_a[:], m_a[:], ACTF.Exp)
        nc.scalar.activation(em_b[:], m_b[:], ACTF.Exp)
        nc.scalar.activation(em_c[:], m_c[:], ACTF.Exp)

        ra = rt.tile([128, NT], F32)
        rb = rt.tile([128, NT], F32)
        rc = rt.tile([128, NT], F32)
        nc.vector.reciprocal(ra[:], sum_a[:])
        nc.vector.reciprocal(rb[:], sum_b[:])
        nc.vector.reciprocal(rc[:], sum_c[:])

        gs = rt.tile([128, NT * 2], F32)
        nc.scalar.activation(
            gs[:].rearrange("p (t g) -> p t g", t=NT), gsc, ACTF.Sigmoid
        )
        g3 = gs[:].rearrange("p (t g) -> p t g", t=NT)

        # gates: gm = (g > THR) * g
        gm0 = rt.tile([128, NT], F32)
        gm1 = rt.tile([128, NT], F32)
        msk = rt.tile([128, NT], F32)
        nc.vector.tensor_scalar(msk[:], g3[:, :, 0:1].rearrange("p t 1 -> p t"), THR, None, op0=OP.is_gt)
        nc.vector.tensor_tensor(gm0[:], msk[:], g3[:, :, 0:1].rearrange("p t 1 -> p t"), op=OP.mult)
        nc.vector.tensor_scalar(msk[:], g3[:, :, 1:2].rearrange("p t 1 -> p t"), THR, None, op0=OP.is_gt)
        nc.vector.tensor_tensor(gm1[:], msk[:], g3[:, :, 1:2].rearrange("p t 1 -> p t"), op=OP.mult)

        # argmax helper: idx = reduce_min( (s<m)*BIG + iota )
        def argmax_idx(out, s_view, m_t, io_view, w, scratch):
            nc.vector.tensor_tensor(scratch, s_view, bcast(m_t, w), op=OP.is_lt)
            nc.vector.scalar_tensor_tensor(
                scratch, scratch, BIG, io_view, op0=OP.mult, op1=OP.add
            )
            nc.vector.tensor_reduce(out, scratch, axis=AX.X, op=OP.min)

        scr8 = rt.tile([128, NT * 8], F32)
        s8 = scr8[:].rearrange("p (t e) -> p t e", t=NT)
        scr4 = rt.tile([128, NT * 4], F32)
        s4 = scr4[:].rearrange("p (t e) -> p t e", t=NT)

        idx_a = rt.tile([128, NT], F32)
        idx_b = rt.tile([128, NT], F32)
        idx_c1 = rt.tile([128, NT], F32)
        idx_c2 = rt.tile([128, NT], F32)
        argmax_idx(idx_a[:], sa, m_a, ioA, 8, s8)
        argmax_idx(idx_b[:], sb4, m_b, ioB, 4, s4)
        argmax_idx(idx_c1[:], sc4, m_c, ioC, 4, s4)

        # mask out top-1 of C by index, find second max
        sc_m = rt.tile([128, NT * 4], F32)
        sm4 = sc_m[:].rearrange("p (t e) -> p t e", t=NT)
        nc.vector.tensor_tensor(s4, ioC, bcast(idx_c1, 4), op=OP.is_equal)
        nc.vector.scalar_tensor_tensor(sm4, s4, -BIG, sc4, op0=OP.mult, op1=OP.add)
        m_c2 = rt.tile([128, NT], F32)
        nc.vector.reduce_max(m_c2[:], sm4, axis=AX.X)
        argmax_idx(idx_c2[:], sm4, m_c2, ioC, 4, s4)
        em_c2 = rt.tile([128, NT], F32)
        nc.scalar.activation(em_c2[:], m_c2[:], ACTF.Exp)

        # ---- weights (128, NT, 6) ----
        W_raw = rt.tile([128, NT * 6], F32)
        W3 = W_raw[:].rearrange("p (t s) -> p t s", t=NT)
        nc.vector.memset(W_raw[:], 0.0)

        def slot(view3, j):
            return view3[:, :, j:j + 1].rearrange("p t 1 -> p t")

        nc.vector.tensor_tensor(slot(W3, 0), em_a[:], ra[:], op=OP.mult)
        tb = rt.tile([128, NT], F32)
        nc.vector.tensor_tensor(tb[:], em_b[:], rb[:], op=OP.mult)
        nc.vector.tensor_tensor(slot(W3, 1), tb[:], gm0[:], op=OP.mult)
        tc1 = rt.tile([128, NT], F32)
        nc.vector.tensor_tensor(tc1[:], em_c[:], rc[:], op=OP.mult)
        nc.vector.tensor_tensor(slot(W3, 2), tc1[:], gm1[:], op=OP.mult)
        nc.vector.tensor_tensor(tc1[:], em_c2[:], rc[:], op=OP.mult)
        nc.vector.tensor_tensor(slot(W3, 3), tc1[:], gm1[:], op=OP.mult)

        sum_w = rt.tile([128, NT], F32)
        nc.vector.reduce_sum(sum_w[:], W3[:, :, 0:4], axis=AX.X)
        nc.vector.tensor_scalar(sum_w[:], sum_w[:], 1e-8, None, op0=OP.add)
        winv = rt.tile([128, NT], F32)
        nc.vector.reciprocal(winv[:], sum_w[:])

        ow_sb = rt.tile([128, NT * 6], F32)
        ow3 = ow_sb[:].rearrange("p (t s) -> p t s", t=NT)
        nc.vector.tensor_tensor(ow3, W3, bcast(winv, 6), op=OP.mult)
        nc.sync.dma_start(ow_d.ap()[:], ow_sb[:])

        # ---- indices (128, NT, 6) int32 ----
        I_f = rt.tile([128, NT * 6], F32)
        I3 = I_f[:].rearrange("p (t s) -> p t s", t=NT)
        nc.vector.memset(I_f[:], 0.0)
        nc.vector.tensor_copy(slot(I3, 0), idx_a[:])
        nc.vector.tensor_copy(slot(I3, 1), idx_b[:])
        nc.vector.tensor_copy(slot(I3, 2), idx_c1[:])
        nc.vector.tensor_copy(slot(I3, 3), idx_c2[:])
        oi_sb = rt.tile([128, NT * 6], I32)
        nc.vector.tensor_copy(oi_sb[:], I_f[:])
        nc.sync.dma_start(oi_d.ap()[:], oi_sb[:])

        # ---- expert counts (pad-slot zeros handled on host) ----
        cnt = rt.tile([128, E], F32)
        cdummy = rt.tile([128, NT * 4], F32)
        cd3 = cdummy[:].rearrange("p (t s) -> p t s", t=NT)
        for e in range(E):
            nc.vector.tensor_scalar(
                cd3, I3[:, :, 0:4], float(e), 0.0, op0=OP.is_equal, op1=OP.add,
                accum_out=cnt[:, e:e + 1],
            )
        nc.sync.dma_start(oc_d.ap()[:], cnt[:])

    nc.compile()
    _cache["nc"] = nc
    return nc


def _run(in_maps, trace=False, tmpdir=None):
    from concourse import bass_utils
    if trace:
        bass_utils.upload_artifacts = lambda d: "local://" + d
    nc = _build()
    return bass_utils.run_bass_kernel_spmd(
        nc, in_maps, core_ids=list(range(N_CORES)), trace=trace, tmpdir=tmpdir
    )


def _make_in_maps(x, W_expert, W_group):
    x = np.ascontiguousarray(np.asarray(x, dtype=np.float32)).reshape(TOK, D)
    W_all = np.concatenate(
        [np.asarray(W_expert, np.float32), np.asarray(W_group, np.float32)], axis=0
    )
    wT = _round_f32r(np.ascontiguousarray(W_all.T))
    ident = np.eye(128, dtype=np.float32)
    iota = np.tile(np.arange(16, dtype=np.float32), (128, 16))
    return [
        {"x": x[c * TPC:(c + 1) * TPC], "wT": wT, "ident": ident, "iota": iota}
        for c in range(N_CORES)
    ]


def _gather(results):
    w_parts, i_parts = [], []
    counts = np.zeros(E, dtype=np.float64)
    for c in range(N_CORES):
        r = results[c]
        w_parts.append(
            r["out_w"].reshape(128, N_TILES, 6).transpose(1, 0, 2).reshape(TPC, 6)
        )
        i_parts.append(
            r["out_i"].reshape(128, N_TILES, 6).transpose(1, 0, 2).reshape(TPC, 6)
        )
        counts += r["out_c"].astype(np.float64).sum(axis=0)
    weights = np.concatenate(w_parts, 0).reshape(B, T, 6).astype(np.float32)
    indices = np.concatenate(i_parts, 0).reshape(B, T, 6).astype(np.int32)
    counts[0] += 2.0 * TOK  # two zero pad slots per token
    actual = counts / counts.sum()
    aux = np.float32(0.01 * np.sum((1.0 / E) * (np.log(1.0 / E) - np.log(actual))))
    return weights, indices, aux


def kernel(x, W_expert, W_group):
    in_maps = _make_in_maps(x, W_expert, W_group)
    res = _run(in_maps)
    return _gather(res.results)


if __name__ == "__main__":
    rng = np.random.default_rng(1)
    x = rng.normal(size=(B, T, D)).astype(np.float32)
    We = (rng.normal(size=(E, D)) * D ** -0.5).astype(np.float32)
    Wg = (rng.normal(size=(2, D)) * D ** -0.5).astype(np.float32)
    w, i, aux = kernel(x, We, Wg)
    print("weights", w.shape, w.dtype, "indices", i.shape, i.dtype, "aux", aux)
